# revision 20
# baseline (speedup 1.0000x reference)
"""Trainium2 Bass kernel for nn_AggressiveLoss (loss_fn over [4096,10,30,30]).

Strategy: pure data parallel over batch B=4096 across 8 NeuronCores (512
samples/core). Each core processes 4 sample-blocks x 3 pixel-chunks as
[128 samples (partitions), 10 channels x 300 pixels (free)] tiles.

Per pixel, the channel argmax is represented as a one-hot plane
E = (x >= max_c x) (bf16: 0/1 exact) built with a strided channel-max reduce
plus one is_ge scalar_tensor_tensor pass whose [P,1] accumulator doubles as a
tie detector. Every per-sample quantity the loss needs is a sum over pixels/
channels of products of these planes; products run as bf16 tensor_tensor (DVE
2x mode) + tensor_scalar row-sum accumulators (4x), the channel-sum of
E_p*E_t runs as a bf16 2x add-tree, per-channel presence counts run on the
otherwise-idle scalar engine (Copy + accum_out), and exp/log run on ACT:

  sum ce*(1+2*inc) = 3*S(logz) - 3*S(x_t) - 2*S(corr*(logz - m_p))
  S(x_t) = S(pred * E_t)   (pred gathered at target argmax)

Per-sample scalars ship to the host ([512,12,32] per core) where the final
few-thousand-element aggregation runs in float64. Pixels with a tied channel
max (argmax ambiguity, ~7 pixels in the whole dataset) are detected per
sample via the free accumulators of the is_ge passes; flagged samples are
recomputed exactly on the host with first-occurrence argmax semantics.
"""

import json

import numpy as np

import concourse.bass as bass
import concourse.mybir as mybir
from concourse.tile import TileContext
from concourse.bass_utils import run_bass_kernel_spmd
from concourse.bass_types import SemaphoreHandle
from concourse.bass import compact_to_ranges

# ---------------------------------------------------------------------------
# Compatibility patches for the walrus build in this container:
#  1. sem_clear(range) emits a raw ISA instruction (opcode 176) that this
#     walrus rejects ("ISA wrong length"); replace with an SWDGE semaphore
#     write of 0 (plain supported path).
#  2. This walrus enforces a tighter per-instruction sync-wait-command limit
#     than Tile's semaphore pass assumes. Post-process the serialized BIR:
#     hoist every instruction's sem waits into standalone single-wait
#     EventSemaphore instructions placed immediately before it on the same
#     engine stream (identical semantics, one wait per instruction).
# ---------------------------------------------------------------------------


def _patched_clear_and_free(self, sems):
    if not sems:
        return
    handles = [s if isinstance(s, SemaphoreHandle) else SemaphoreHandle(f"s{s}", s)
               for s in sems]
    sem_nums = [h.num for h in handles]
    for sem_range in compact_to_ranges(sem_nums):
        assert self._state.free_isdisjoint(sem_range)
        self.gpsimd.dma_reset(sem_range)
    self.gpsimd.inc_swdge_sem(handles, [0] * len(handles), mode="wr")
    self._state.prepend_free_semaphores(sem_nums)
    for poison_set in self._tile_sem_poison_stack:
        poison_set.update(sem_nums)


bass.Bass.clear_and_free_semaphores = _patched_clear_and_free

_orig_to_json_bytes = bass.Bass.to_json_bytes


def _hoist_waits_to_json_bytes(self):
    raw = _orig_to_json_bytes(self)
    m = json.loads(raw)
    ctr = 0
    for f in m.get("functions", []):
        for bb in f.get("blocks", []):
            new_insts = []
            for inst in bb.get("instructions", []):
                if (inst.get("opcode") == "ISA"
                        and inst.get("op_name") == "InstIncSwdgeSem"
                        and inst.get("mode") == "wr"):
                    # rewrite raw sem-write ISA op into per-sem EventSemaphore
                    base = inst.get("sem_id_base", 0)
                    names = inst.get("sem_names", [])
                    values = inst.get("sem_values", [])
                    for k, (nm, val) in enumerate(zip(names, values)):
                        ctr += 1
                        new_insts.append({
                            "debug": inst.get("debug", 0),
                            "engine": inst.get("engine"),
                            "ins": [], "outs": [],
                            "name": f"semwr_{ctr}_{inst.get('name')}",
                            "opcode": "EventSemaphore",
                            "sync_info": {"on_update": [{
                                "ant_name": nm, "id": base + k,
                                "sync_type": "semaphore",
                                "update_mode": "sem-wr-imm",
                                "update_value": int(val)}],
                                "on_wait": []},
                        })
                    continue
                si = inst.get("sync_info")
                waits = (si or {}).get("on_wait") or []
                if len(waits) > 1:
                    for w in waits:
                        ctr += 1
                        new_insts.append({
                            "debug": inst.get("debug", 0),
                            "engine": inst.get("engine"),
                            "ins": [], "outs": [],
                            "name": f"hoistw_{ctr}_{inst.get('name')}",
                            "opcode": "EventSemaphore",
                            "sync_info": {"on_update": [], "on_wait": [w]},
                        })
                    si["on_wait"] = []
                new_insts.append(inst)
            bb["instructions"] = new_insts
    return json.dumps(m).encode()


bass.Bass.to_json_bytes = _hoist_waits_to_json_bytes

F32 = mybir.dt.float32
BF16 = mybir.dt.bfloat16
ALU = mybir.AluOpType
ACT = mybir.ActivationFunctionType
AX = mybir.AxisListType

B, C, H, W = 4096, 10, 30, 30
HW = H * W                      # 900
NCORES = 8
BS = B // NCORES                # 512 samples per core
P = 128                         # partitions (samples per block)
NBLK = BS // P                  # 4 sample blocks
NCH = 3                         # pixel chunks per plane
PX = HW // NCH                  # 300 pixels per chunk
NSUB = NBLK * NCH               # 12 subtiles
NST = 32                        # stats columns per subtile

# stats column layout
S_CNTP, S_CNTT, S_CNTI = 0, 1, 2
S_PT, S_PI, S_TI = 3, 4, 5
S_XT, S_LOGZ, S_CLOGZ, S_CMP = 6, 7, 8, 9
S_NP, S_NT = 10, 20             # 10 columns each


def _bcast_c(ap):
    """View a [P, PX] AP as [P, C, PX] with a step-0 broadcast over channels."""
    return bass.AP(tensor=ap.tensor, offset=ap.offset,
                   ap=[ap.ap[0], [0, C], ap.ap[1]])


def build_bass(use_gpsimd=False, bufs_inp=2, bufs_work=2, bufs_small=2,
               expb_psum=False, skip=()):
    skip = set(skip)
    nc = bass.Bass()
    pred_d = nc.dram_tensor("pred", [BS, C * HW], F32, kind="ExternalInput")
    targ_d = nc.dram_tensor("target", [BS, C * HW], F32, kind="ExternalInput")
    inp_d = nc.dram_tensor("input_grid", [BS, C * HW], F32, kind="ExternalInput")
    out_d = nc.dram_tensor("stats", [BS, NSUB, NST], F32, kind="ExternalOutput")

    drams = [d.rearrange("n (c k x) -> n c k x", c=C, k=NCH)
             for d in (pred_d, targ_d, inp_d)]

    with TileContext(nc) as tc:
        with (
            tc.tile_pool(name="inp", bufs=bufs_inp) as inp_pool,
            tc.tile_pool(name="work", bufs=bufs_work) as work_pool,
            tc.tile_pool(name="one", bufs=1) as one_pool,
            tc.tile_pool(name="small", bufs=bufs_small) as small_pool,
            tc.tile_pool(name="psum", bufs=1, space="PSUM") as psum_pool,
        ):
            for blk in range(NBLK):
                rows = slice(blk * P, (blk + 1) * P)
                for ch in range(NCH):
                    sub = blk * NCH + ch
                    # ---- loads ----
                    pred = inp_pool.tile([P, C * PX], F32, tag="pred")
                    targ = inp_pool.tile([P, C * PX], F32, tag="targ")
                    igrd = inp_pool.tile([P, C * PX], F32, tag="igrd")
                    for t, d in zip((pred, targ, igrd), drams):
                        nc.sync.dma_start(
                            out=t[:].rearrange("p (c x) -> p c x", c=C),
                            in_=d[rows, :, ch, :])

                    st = small_pool.tile([P, NST], F32, tag="st")
                    mp = small_pool.tile([P, PX], F32, tag="mp")
                    mt = small_pool.tile([P, PX], F32, tag="mt")
                    mi = small_pool.tile([P, PX], F32, tag="mi")

                    def cx(t):     # [P, C, PX] natural view
                        return t[:].rearrange("p (c x) -> p c x", c=C)

                    def xc(t):     # [P, PX, C] channel-innermost view
                        return t[:].rearrange("p (c x) -> p x c", c=C)

                    # ---- channel max per pixel ----
                    if "max" not in skip:
                        for m, t in ((mp, pred), (mt, targ), (mi, igrd)):
                            nc.vector.tensor_reduce(out=m[:], in_=xc(t),
                                                    axis=AX.X, op=ALU.max)

                    # exp(pred) on ACT in parallel with DVE work
                    if expb_psum:
                        expb = psum_pool.tile([P, C * PX], F32, tag="expb")
                    else:
                        expb = work_pool.tile([P, C * PX], F32, tag="expb")
                    nc.scalar.activation(out=expb[:], in_=pred[:], func=ACT.Exp)

                    # ---- one-hot argmax planes in bf16 (+ tie counters) ----
                    # 0/1 indicators are exact in bf16 and unlock the DVE 2x
                    # perf mode for the pairwise product passes below.
                    Ep = work_pool.tile([P, C * PX], BF16, tag="Ep")
                    Et = work_pool.tile([P, C * PX], BF16, tag="Et")
                    Ei = one_pool.tile([P, C * PX], BF16, tag="Ei")
                    if "isge" not in skip:
                        for E, t, m, col in ((Ep, pred, mp, S_CNTP),
                                             (Et, targ, mt, S_CNTT),
                                             (Ei, igrd, mi, S_CNTI)):
                            nc.vector.scalar_tensor_tensor(
                                out=cx(E), in0=cx(t), scalar=1.0,
                                in1=_bcast_c(m[:]),
                                op0=ALU.mult, op1=ALU.is_ge,
                                accum_out=st[:, col:col + 1])

                    # ---- z = sum_c exp, logz (+ per-sample sum logz) ----
                    z = small_pool.tile([P, PX], F32, tag="z")
                    if "z" not in skip:
                        nc.vector.tensor_reduce(out=z[:], in_=xc(expb),
                                                axis=AX.X, op=ALU.add)
                    logz = small_pool.tile([P, PX], F32, tag="logz")
                    nc.scalar.activation(out=logz[:], in_=z[:], func=ACT.Ln,
                                         accum_out=st[:, S_LOGZ:S_LOGZ + 1])

    # ---- pairwise products ----
                    # bf16 TT runs in the DVE 2x mode; bf16 TS+accum runs 4x.
                    # STT has no perf modes, so TT+TS beats a fused STT here.
                    eptb = work_pool.tile([P, C * PX], BF16, tag="eptb")
                    junkb = one_pool.tile([P, C * PX], BF16, tag="junkb")
                    junkc = one_pool.tile([P, C * PX], BF16, tag="junkc")
                    if "prod" not in skip:
                        nc.vector.tensor_tensor(out=eptb[:], in0=Ep[:],
                                                in1=Et[:], op=ALU.mult)
                        nc.vector.tensor_tensor(out=junkb[:], in0=Ep[:],
                                                in1=Ei[:], op=ALU.mult)
                        nc.vector.tensor_scalar(
                            out=junkc[:], in0=junkb[:], scalar1=1.0,
                            scalar2=0.0, op0=ALU.mult, op1=ALU.add,
                            accum_out=st[:, S_PI:S_PI + 1])
                        nc.vector.tensor_tensor(out=junkb[:], in0=Et[:],
                                                in1=Ei[:], op=ALU.mult)
                        nc.vector.tensor_scalar(
                            out=junkc[:], in0=junkb[:], scalar1=1.0,
                            scalar2=0.0, op0=ALU.mult, op1=ALU.add,
                            accum_out=st[:, S_TI:S_TI + 1])

                    # sum of pred gathered at target argmax (f32 out/accum);
                    # expb is dead after the z reduce, reuse it as scratch
                    if "xt" not in skip:
                        nc.vector.scalar_tensor_tensor(
                            out=expb[:], in0=pred[:], scalar=1.0, in1=Et[:],
                            op0=ALU.mult, op1=ALU.mult,
                            accum_out=st[:, S_XT:S_XT + 1])

                    # corr per pixel = sum_c Ep*Et via a bf16 2x add-tree
                    # (tensor_reduce only has a 1x uop); values stay 0/1 so
                    # bf16 is exact. 10 -> 5 -> (2x2 + carry) -> 1
                    treeb = small_pool.tile([P, 5 * PX], BF16, tag="treeb")
                    tree2 = small_pool.tile([P, 2 * PX], BF16, tag="tree2")
                    tree3 = small_pool.tile([P, PX], BF16, tag="tree3")
                    corr = small_pool.tile([P, PX], BF16, tag="corr")
                    if "corr" not in skip:
                        e5 = cx(eptb)
                        t5 = treeb[:].rearrange("p (c x) -> p c x", c=5)
                        nc.vector.tensor_tensor(out=t5, in0=e5[:, 0:5, :],
                                                in1=e5[:, 5:10, :], op=ALU.add)
                        nc.vector.tensor_tensor(
                            out=tree2[:].rearrange("p (c x) -> p c x", c=2),
                            in0=t5[:, 0:2, :], in1=t5[:, 2:4, :], op=ALU.add)
                        nc.vector.tensor_tensor(out=tree3[:],
                                                in0=tree2[:, 0:PX],
                                                in1=tree2[:, PX:2 * PX],
                                                op=ALU.add)
                        nc.vector.tensor_tensor(out=corr[:], in0=tree3[:],
                                                in1=t5[:, 4, :], op=ALU.add)
                        # s_pt = row-sum of corr (bf16 TS 4x)
                        nc.vector.tensor_scalar(
                            out=tree3[:], in0=corr[:], scalar1=1.0,
                            scalar2=0.0, op0=ALU.mult, op1=ALU.add,
                            accum_out=st[:, S_PT:S_PT + 1])

                    # per-channel presence counts on the (mostly idle) ACT
                    # engine: Copy with accum_out per channel
                    junkA = small_pool.tile([P, PX], F32, tag="junkA")
                    if "pres" not in skip:
                        for c in range(C):
                            nc.scalar.activation(
                                out=junkA[:], in_=cx(Ep)[:, c, :], func=ACT.Copy,
                                accum_out=st[:, S_NP + c:S_NP + c + 1])
                            nc.scalar.activation(
                                out=junkA[:], in_=cx(Et)[:, c, :], func=ACT.Copy,
                                accum_out=st[:, S_NT + c:S_NT + c + 1])

                    # coupling: sum corr*(logz - m_p)  (stored in S_CLOGZ)
                    d = small_pool.tile([P, PX], F32, tag="d")
                    nc.vector.scalar_tensor_tensor(
                        out=d[:], in0=mp[:], scalar=-1.0, in1=logz[:],
                        op0=ALU.mult, op1=ALU.add)
                    j3 = small_pool.tile([P, PX], F32, tag="j3")
                    nc.vector.scalar_tensor_tensor(
                        out=j3[:], in0=corr[:], scalar=1.0, in1=d[:],
                        op0=ALU.mult, op1=ALU.mult,
                        accum_out=st[:, S_CLOGZ:S_CLOGZ + 1])

                    nc.sync.dma_start(out=out_d[rows, sub, :], in_=st[:])
    return nc


def _run_device(inputs, trace=False, use_gpsimd=False):
    nc = build_bass(use_gpsimd=use_gpsimd)
    flat = {k: np.ascontiguousarray(np.asarray(v), dtype=np.float32)
            .reshape(B, C * HW) for k, v in inputs.items()}
    in_maps = [{k: v[i * BS:(i + 1) * BS] for k, v in flat.items()}
               for i in range(NCORES)]
    res = run_bass_kernel_spmd(nc, in_maps, core_ids=list(range(NCORES)),
                               trace=trace)
    stats = np.concatenate([r["stats"] for r in res.results], axis=0)
    return stats, res


def _exact_sample(pred_b, targ_b, inp_b):
    """Exact per-sample quantities (first-occurrence argmax), numpy f64.
    pred_b/targ_b/inp_b: [C, HW] float32."""
    pi = pred_b.argmax(0)
    ti = targ_b.argmax(0)
    ii = inp_b.argmax(0)
    p64 = pred_b.astype(np.float64)
    logz = np.log(np.exp(p64).sum(0))
    xt = np.take_along_axis(p64, ti[None], 0)[0]
    ce = logz - xt
    corr = (pi == ti)
    sum_ce_w = (ce * (3.0 - 2.0 * corr)).sum()
    exact_b = float(corr.all())
    should_b = float((ti != ii).any())
    did_b = float((pi == ii).all())
    changed = (pi != ii).mean()
    t_changed = (ti != ii).mean()
    td_b = (changed - t_changed) ** 2
    pres_p = np.zeros(C, bool)
    pres_t = np.zeros(C, bool)
    pres_p[np.unique(pi)] = True
    pres_t[np.unique(ti)] = True
    missing_b = float((pres_t & ~pres_p).sum())
    return sum_ce_w, exact_b, should_b, did_b, td_b, missing_b


def _aggregate(stats, inputs):
    """stats: [B, NSUB, NST] f32 -> loss tuple (f64 internally)."""
    s = stats.astype(np.float64)
    tot = s.sum(axis=1)                      # [B, NST] additive columns
    n_corr = tot[:, S_PT]
    n_pi = tot[:, S_PI]
    n_ti = tot[:, S_TI]
    # S_CLOGZ holds sum corr*(logz - m_p); for correct pixels x_t = m_p, so
    # sum ce*(1+2inc) = 3*S(logz) - 3*S(x_t) - 2*S(corr*(logz - m_p))
    sum_ce_w = (3.0 * tot[:, S_LOGZ] - 3.0 * tot[:, S_XT]
                - 2.0 * tot[:, S_CLOGZ])
    exact_b = (n_corr > HW - 0.5).astype(np.float64)
    should_b = (n_ti < HW - 0.5).astype(np.float64)
    did_b = (n_pi > HW - 0.5).astype(np.float64)
    changed = (HW - n_pi) / HW
    t_changed = (HW - n_ti) / HW
    td_b = (changed - t_changed) ** 2
    pres_p = tot[:, S_NP:S_NP + C] > 0.5
    pres_t = tot[:, S_NT:S_NT + C] > 0.5
    missing_b = (pres_t & ~pres_p).sum(axis=1).astype(np.float64)

    # tie-flagged samples: recompute exactly on host
    cnt = tot[:, S_CNTP] + tot[:, S_CNTT] + tot[:, S_CNTI]
    flagged = np.nonzero(cnt > 3 * HW + 0.5)[0]
    if len(flagged):
        pr = inputs["pred"].reshape(B, C, HW)
        tg = inputs["target"].reshape(B, C, HW)
        ig = inputs["input_grid"].reshape(B, C, HW)
        for b in flagged:
            (sum_ce_w[b], exact_b[b], should_b[b], did_b[b],
             td_b[b], missing_b[b]) = _exact_sample(pr[b], tg[b], ig[b])

    ce_loss = sum_ce_w.sum() / (B * HW) + 0.5 * missing_b.sum()
    exact_bonus = -10.0 * exact_b.mean()
    copy_penalty = 5.0 * (should_b * did_b).mean()
    transform_diff = 2.0 * td_b.mean()
    total = ce_loss + exact_bonus + copy_penalty + transform_diff
    return (np.float32(total), np.float32(ce_loss), np.float32(exact_bonus),
            np.float32(copy_penalty), np.float32(transform_diff),
            np.float32(exact_b.sum()))


def kernel(pred, target, input_grid):
    inputs = {"pred": np.ascontiguousarray(np.asarray(pred), dtype=np.float32),
              "target": np.ascontiguousarray(np.asarray(target), dtype=np.float32),
              "input_grid": np.ascontiguousarray(np.asarray(input_grid),
                                                 dtype=np.float32)}
    stats, _ = _run_device(inputs, trace=False)
    return _aggregate(stats, inputs)


# revision 21
# speedup vs baseline: 1.0120x; 1.0120x over previous
"""Trainium2 Bass kernel for nn_AggressiveLoss (loss_fn over [4096,10,30,30]).

Strategy: pure data parallel over batch B=4096 across 8 NeuronCores (512
samples/core). Each core processes 4 sample-blocks x 3 pixel-chunks as
[128 samples (partitions), 10 channels x 300 pixels (free)] tiles.

Per pixel, the channel argmax is represented as a one-hot plane
E = (x >= max_c x) (bf16: 0/1 exact) built with a strided channel-max reduce
plus one is_ge scalar_tensor_tensor pass whose [P,1] accumulator doubles as a
tie detector. Every per-sample quantity the loss needs is a sum over pixels/
channels of products of these planes; products run as bf16 tensor_tensor (DVE
2x mode) + tensor_scalar row-sum accumulators (4x), the channel-sum of
E_p*E_t runs as a bf16 2x add-tree, per-channel presence counts run on the
otherwise-idle scalar engine (Copy + accum_out), and exp/log run on ACT:

  sum ce*(1+2*inc) = 3*S(logz) - 3*S(x_t) - 2*S(corr*(logz - m_p))
  S(x_t) = S(pred * E_t)   (pred gathered at target argmax)

Per-sample scalars ship to the host ([512,12,32] per core) where the final
few-thousand-element aggregation runs in float64. Pixels with a tied channel
max (argmax ambiguity, ~7 pixels in the whole dataset) are detected per
sample via the free accumulators of the is_ge passes; flagged samples are
recomputed exactly on the host with first-occurrence argmax semantics.
"""

import json

import numpy as np

import concourse.bass as bass
import concourse.mybir as mybir
from concourse.tile import TileContext
from concourse.bass_utils import run_bass_kernel_spmd
from concourse.bass_types import SemaphoreHandle
from concourse.bass import compact_to_ranges

# ---------------------------------------------------------------------------
# Compatibility patches for the walrus build in this container:
#  1. sem_clear(range) emits a raw ISA instruction (opcode 176) that this
#     walrus rejects ("ISA wrong length"); replace with an SWDGE semaphore
#     write of 0 (plain supported path).
#  2. This walrus enforces a tighter per-instruction sync-wait-command limit
#     than Tile's semaphore pass assumes. Post-process the serialized BIR:
#     hoist every instruction's sem waits into standalone single-wait
#     EventSemaphore instructions placed immediately before it on the same
#     engine stream (identical semantics, one wait per instruction).
# ---------------------------------------------------------------------------


def _patched_clear_and_free(self, sems):
    if not sems:
        return
    handles = [s if isinstance(s, SemaphoreHandle) else SemaphoreHandle(f"s{s}", s)
               for s in sems]
    sem_nums = [h.num for h in handles]
    for sem_range in compact_to_ranges(sem_nums):
        assert self._state.free_isdisjoint(sem_range)
        self.gpsimd.dma_reset(sem_range)
    self.gpsimd.inc_swdge_sem(handles, [0] * len(handles), mode="wr")
    self._state.prepend_free_semaphores(sem_nums)
    for poison_set in self._tile_sem_poison_stack:
        poison_set.update(sem_nums)


bass.Bass.clear_and_free_semaphores = _patched_clear_and_free

_orig_to_json_bytes = bass.Bass.to_json_bytes


def _hoist_waits_to_json_bytes(self):
    raw = _orig_to_json_bytes(self)
    m = json.loads(raw)
    ctr = 0
    for f in m.get("functions", []):
        for bb in f.get("blocks", []):
            new_insts = []
            for inst in bb.get("instructions", []):
                if (inst.get("opcode") == "ISA"
                        and inst.get("op_name") == "InstIncSwdgeSem"
                        and inst.get("mode") == "wr"):
                    # rewrite raw sem-write ISA op into per-sem EventSemaphore
                    base = inst.get("sem_id_base", 0)
                    names = inst.get("sem_names", [])
                    values = inst.get("sem_values", [])
                    for k, (nm, val) in enumerate(zip(names, values)):
                        ctr += 1
                        new_insts.append({
                            "debug": inst.get("debug", 0),
                            "engine": inst.get("engine"),
                            "ins": [], "outs": [],
                            "name": f"semwr_{ctr}_{inst.get('name')}",
                            "opcode": "EventSemaphore",
                            "sync_info": {"on_update": [{
                                "ant_name": nm, "id": base + k,
                                "sync_type": "semaphore",
                                "update_mode": "sem-wr-imm",
                                "update_value": int(val)}],
                                "on_wait": []},
                        })
                    continue
                si = inst.get("sync_info")
                waits = (si or {}).get("on_wait") or []
                if len(waits) > 1:
                    for w in waits:
                        ctr += 1
                        new_insts.append({
                            "debug": inst.get("debug", 0),
                            "engine": inst.get("engine"),
                            "ins": [], "outs": [],
                            "name": f"hoistw_{ctr}_{inst.get('name')}",
                            "opcode": "EventSemaphore",
                            "sync_info": {"on_update": [], "on_wait": [w]},
                        })
                    si["on_wait"] = []
                new_insts.append(inst)
            bb["instructions"] = new_insts
    return json.dumps(m).encode()


bass.Bass.to_json_bytes = _hoist_waits_to_json_bytes

F32 = mybir.dt.float32
BF16 = mybir.dt.bfloat16
ALU = mybir.AluOpType
ACT = mybir.ActivationFunctionType
AX = mybir.AxisListType

B, C, H, W = 4096, 10, 30, 30
HW = H * W                      # 900
NCORES = 8
BS = B // NCORES                # 512 samples per core
P = 128                         # partitions (samples per block)
NBLK = BS // P                  # 4 sample blocks
NCH = 3                         # pixel chunks per plane
PX = HW // NCH                  # 300 pixels per chunk
NSUB = NBLK * NCH               # 12 subtiles
NST = 32                        # stats columns per subtile

# stats column layout
S_CNTP, S_CNTT, S_CNTI = 0, 1, 2
S_PT, S_PI, S_TI = 3, 4, 5
S_XT, S_LOGZ, S_CLOGZ, S_CMP = 6, 7, 8, 9
S_NP, S_NT = 10, 20             # 10 columns each


def _bcast_c(ap):
    """View a [P, PX] AP as [P, C, PX] with a step-0 broadcast over channels."""
    return bass.AP(tensor=ap.tensor, offset=ap.offset,
                   ap=[ap.ap[0], [0, C], ap.ap[1]])


def build_bass(use_gpsimd=False, bufs_inp=2, bufs_work=2, bufs_small=2,
               expb_psum=False, skip=()):
    skip = set(skip)
    nc = bass.Bass()
    pred_d = nc.dram_tensor("pred", [BS, C * HW], F32, kind="ExternalInput")
    targ_d = nc.dram_tensor("target", [BS, C * HW], F32, kind="ExternalInput")
    inp_d = nc.dram_tensor("input_grid", [BS, C * HW], F32, kind="ExternalInput")
    out_d = nc.dram_tensor("stats", [BS, NSUB, NST], F32, kind="ExternalOutput")

    drams = [d.rearrange("n (c k x) -> n c k x", c=C, k=NCH)
             for d in (pred_d, targ_d, inp_d)]

    with TileContext(nc) as tc:
        with (
            tc.tile_pool(name="inp", bufs=bufs_inp) as inp_pool,
            tc.tile_pool(name="work", bufs=bufs_work) as work_pool,
            tc.tile_pool(name="one", bufs=1) as one_pool,
            tc.tile_pool(name="small", bufs=bufs_small) as small_pool,
            tc.tile_pool(name="psum", bufs=1, space="PSUM") as psum_pool,
        ):
            for blk in range(NBLK):
                rows = slice(blk * P, (blk + 1) * P)
                for ch in range(NCH):
                    sub = blk * NCH + ch
                    # ---- loads ----
                    pred = inp_pool.tile([P, C * PX], F32, tag="pred")
                    targ = inp_pool.tile([P, C * PX], F32, tag="targ")
                    igrd = inp_pool.tile([P, C * PX], F32, tag="igrd")
                    for t, d in zip((pred, targ, igrd), drams):
                        nc.sync.dma_start(
                            out=t[:].rearrange("p (c x) -> p c x", c=C),
                            in_=d[rows, :, ch, :])

                    st = small_pool.tile([P, NST], F32, tag="st")
                    mp = small_pool.tile([P, PX], F32, tag="mp")
                    mt = small_pool.tile([P, PX], F32, tag="mt")
                    mi = small_pool.tile([P, PX], F32, tag="mi")

                    def cx(t):     # [P, C, PX] natural view
                        return t[:].rearrange("p (c x) -> p c x", c=C)

                    def xc(t):     # [P, PX, C] channel-innermost view
                        return t[:].rearrange("p (c x) -> p x c", c=C)

                    # ---- channel max per pixel (pairwise f32 tree:
                    # 10 -> 5 -> (2x2 + carry) -> 1; slightly cheaper than
                    # the 1x-uop strided tensor_reduce) ----
                    ft5 = small_pool.tile([P, 5 * PX], F32, tag="ft5")
                    ft2 = small_pool.tile([P, 2 * PX], F32, tag="ft2")
                    ft1 = small_pool.tile([P, PX], F32, tag="ft1")
                    if "max" not in skip:
                        for m, t in ((mp, pred), (mt, targ), (mi, igrd)):
                            v = cx(t)
                            t5v = ft5[:].rearrange("p (c x) -> p c x", c=5)
                            nc.vector.tensor_tensor(
                                out=t5v, in0=v[:, 0:5, :], in1=v[:, 5:10, :],
                                op=ALU.max)
                            nc.vector.tensor_tensor(
                                out=ft2[:].rearrange("p (c x) -> p c x", c=2),
                                in0=t5v[:, 0:2, :], in1=t5v[:, 2:4, :],
                                op=ALU.max)
                            nc.vector.tensor_tensor(
                                out=ft1[:], in0=ft2[:, 0:PX],
                                in1=ft2[:, PX:2 * PX], op=ALU.max)
                            nc.vector.tensor_tensor(
                                out=m[:], in0=ft1[:], in1=t5v[:, 4, :],
                                op=ALU.max)

                    # exp(pred) on ACT in parallel with DVE work
                    if expb_psum:
                        expb = psum_pool.tile([P, C * PX], F32, tag="expb")
                    else:
                        expb = work_pool.tile([P, C * PX], F32, tag="expb")
                    nc.scalar.activation(out=expb[:], in_=pred[:], func=ACT.Exp)

                    # ---- one-hot argmax planes in bf16 (+ tie counters) ----
                    # 0/1 indicators are exact in bf16 and unlock the DVE 2x
                    # perf mode for the pairwise product passes below.
                    Ep = work_pool.tile([P, C * PX], BF16, tag="Ep")
                    Et = work_pool.tile([P, C * PX], BF16, tag="Et")
                    Ei = one_pool.tile([P, C * PX], BF16, tag="Ei")
                    if "isge" not in skip:
                        for E, t, m, col in ((Ep, pred, mp, S_CNTP),
                                             (Et, targ, mt, S_CNTT),
                                             (Ei, igrd, mi, S_CNTI)):
                            nc.vector.scalar_tensor_tensor(
                                out=cx(E), in0=cx(t), scalar=1.0,
                                in1=_bcast_c(m[:]),
                                op0=ALU.mult, op1=ALU.is_ge,
                                accum_out=st[:, col:col + 1])

                    # ---- z = sum_c exp, logz (+ per-sample sum logz) ----
                    z = small_pool.tile([P, PX], F32, tag="z")
                    if "z" not in skip:
                        ev = cx(expb)
                        t5v = ft5[:].rearrange("p (c x) -> p c x", c=5)
                        nc.vector.tensor_tensor(
                            out=t5v, in0=ev[:, 0:5, :], in1=ev[:, 5:10, :],
                            op=ALU.add)
                        nc.vector.tensor_tensor(
                            out=ft2[:].rearrange("p (c x) -> p c x", c=2),
                            in0=t5v[:, 0:2, :], in1=t5v[:, 2:4, :], op=ALU.add)
                        nc.vector.tensor_tensor(
                            out=ft1[:], in0=ft2[:, 0:PX], in1=ft2[:, PX:2 * PX],
                            op=ALU.add)
                        nc.vector.tensor_tensor(
                            out=z[:], in0=ft1[:], in1=t5v[:, 4, :], op=ALU.add)
                    logz = small_pool.tile([P, PX], F32, tag="logz")
                    nc.scalar.activation(out=logz[:], in_=z[:], func=ACT.Ln,
                                         accum_out=st[:, S_LOGZ:S_LOGZ + 1])

    # ---- pairwise products ----
                    # bf16 TT runs in the DVE 2x mode; bf16 TS+accum runs 4x.
                    # STT has no perf modes, so TT+TS beats a fused STT here.
                    eptb = work_pool.tile([P, C * PX], BF16, tag="eptb")
                    junkb = one_pool.tile([P, C * PX], BF16, tag="junkb")
                    junkc = one_pool.tile([P, C * PX], BF16, tag="junkc")
                    if "prod" not in skip:
                        nc.vector.tensor_tensor(out=eptb[:], in0=Ep[:],
                                                in1=Et[:], op=ALU.mult)
                        nc.vector.tensor_tensor(out=junkb[:], in0=Ep[:],
                                                in1=Ei[:], op=ALU.mult)
                        nc.vector.tensor_scalar(
                            out=junkc[:], in0=junkb[:], scalar1=1.0,
                            scalar2=0.0, op0=ALU.mult, op1=ALU.add,
                            accum_out=st[:, S_PI:S_PI + 1])
                        nc.vector.tensor_tensor(out=junkb[:], in0=Et[:],
                                                in1=Ei[:], op=ALU.mult)
                        nc.vector.tensor_scalar(
                            out=junkc[:], in0=junkb[:], scalar1=1.0,
                            scalar2=0.0, op0=ALU.mult, op1=ALU.add,
                            accum_out=st[:, S_TI:S_TI + 1])

                    # sum of pred gathered at target argmax (f32 out/accum);
                    # expb is dead after the z reduce, reuse it as scratch
                    if "xt" not in skip:
                        nc.vector.scalar_tensor_tensor(
                            out=expb[:], in0=pred[:], scalar=1.0, in1=Et[:],
                            op0=ALU.mult, op1=ALU.mult,
                            accum_out=st[:, S_XT:S_XT + 1])

                    # corr per pixel = sum_c Ep*Et via a bf16 2x add-tree
                    # (tensor_reduce only has a 1x uop); values stay 0/1 so
                    # bf16 is exact. 10 -> 5 -> (2x2 + carry) -> 1
                    treeb = small_pool.tile([P, 5 * PX], BF16, tag="treeb")
                    tree2 = small_pool.tile([P, 2 * PX], BF16, tag="tree2")
                    tree3 = small_pool.tile([P, PX], BF16, tag="tree3")
                    corr = small_pool.tile([P, PX], BF16, tag="corr")
                    if "corr" not in skip:
                        e5 = cx(eptb)
                        t5 = treeb[:].rearrange("p (c x) -> p c x", c=5)
                        nc.vector.tensor_tensor(out=t5, in0=e5[:, 0:5, :],
                                                in1=e5[:, 5:10, :], op=ALU.add)
                        nc.vector.tensor_tensor(
                            out=tree2[:].rearrange("p (c x) -> p c x", c=2),
                            in0=t5[:, 0:2, :], in1=t5[:, 2:4, :], op=ALU.add)
                        nc.vector.tensor_tensor(out=tree3[:],
                                                in0=tree2[:, 0:PX],
                                                in1=tree2[:, PX:2 * PX],
                                                op=ALU.add)
                        nc.vector.tensor_tensor(out=corr[:], in0=tree3[:],
                                                in1=t5[:, 4, :], op=ALU.add)
                        # s_pt = row-sum of corr (bf16 TS 4x)
                        nc.vector.tensor_scalar(
                            out=tree3[:], in0=corr[:], scalar1=1.0,
                            scalar2=0.0, op0=ALU.mult, op1=ALU.add,
                            accum_out=st[:, S_PT:S_PT + 1])

                    # per-channel presence counts on the (mostly idle) ACT
                    # engine: Copy with accum_out per channel
                    junkA = small_pool.tile([P, PX], F32, tag="junkA")
                    if "pres" not in skip:
                        for c in range(C):
                            nc.scalar.activation(
                                out=junkA[:], in_=cx(Ep)[:, c, :], func=ACT.Copy,
                                accum_out=st[:, S_NP + c:S_NP + c + 1])
                            nc.scalar.activation(
                                out=junkA[:], in_=cx(Et)[:, c, :], func=ACT.Copy,
                                accum_out=st[:, S_NT + c:S_NT + c + 1])

                    # coupling: sum corr*(logz - m_p)  (stored in S_CLOGZ)
                    d = small_pool.tile([P, PX], F32, tag="d")
                    nc.vector.scalar_tensor_tensor(
                        out=d[:], in0=mp[:], scalar=-1.0, in1=logz[:],
                        op0=ALU.mult, op1=ALU.add)
                    j3 = small_pool.tile([P, PX], F32, tag="j3")
                    nc.vector.scalar_tensor_tensor(
                        out=j3[:], in0=corr[:], scalar=1.0, in1=d[:],
                        op0=ALU.mult, op1=ALU.mult,
                        accum_out=st[:, S_CLOGZ:S_CLOGZ + 1])

                    nc.sync.dma_start(out=out_d[rows, sub, :], in_=st[:])
    return nc


def _run_device(inputs, trace=False, use_gpsimd=False):
    nc = build_bass(use_gpsimd=use_gpsimd)
    flat = {k: np.ascontiguousarray(np.asarray(v), dtype=np.float32)
            .reshape(B, C * HW) for k, v in inputs.items()}
    in_maps = [{k: v[i * BS:(i + 1) * BS] for k, v in flat.items()}
               for i in range(NCORES)]
    res = run_bass_kernel_spmd(nc, in_maps, core_ids=list(range(NCORES)),
                               trace=trace)
    stats = np.concatenate([r["stats"] for r in res.results], axis=0)
    return stats, res


def _exact_sample(pred_b, targ_b, inp_b):
    """Exact per-sample quantities (first-occurrence argmax), numpy f64.
    pred_b/targ_b/inp_b: [C, HW] float32."""
    pi = pred_b.argmax(0)
    ti = targ_b.argmax(0)
    ii = inp_b.argmax(0)
    p64 = pred_b.astype(np.float64)
    logz = np.log(np.exp(p64).sum(0))
    xt = np.take_along_axis(p64, ti[None], 0)[0]
    ce = logz - xt
    corr = (pi == ti)
    sum_ce_w = (ce * (3.0 - 2.0 * corr)).sum()
    exact_b = float(corr.all())
    should_b = float((ti != ii).any())
    did_b = float((pi == ii).all())
    changed = (pi != ii).mean()
    t_changed = (ti != ii).mean()
    td_b = (changed - t_changed) ** 2
    pres_p = np.zeros(C, bool)
    pres_t = np.zeros(C, bool)
    pres_p[np.unique(pi)] = True
    pres_t[np.unique(ti)] = True
    missing_b = float((pres_t & ~pres_p).sum())
    return sum_ce_w, exact_b, should_b, did_b, td_b, missing_b


def _aggregate(stats, inputs):
    """stats: [B, NSUB, NST] f32 -> loss tuple (f64 internally)."""
    s = stats.astype(np.float64)
    tot = s.sum(axis=1)                      # [B, NST] additive columns
    n_corr = tot[:, S_PT]
    n_pi = tot[:, S_PI]
    n_ti = tot[:, S_TI]
    # S_CLOGZ holds sum corr*(logz - m_p); for correct pixels x_t = m_p, so
    # sum ce*(1+2inc) = 3*S(logz) - 3*S(x_t) - 2*S(corr*(logz - m_p))
    sum_ce_w = (3.0 * tot[:, S_LOGZ] - 3.0 * tot[:, S_XT]
                - 2.0 * tot[:, S_CLOGZ])
    exact_b = (n_corr > HW - 0.5).astype(np.float64)
    should_b = (n_ti < HW - 0.5).astype(np.float64)
    did_b = (n_pi > HW - 0.5).astype(np.float64)
    changed = (HW - n_pi) / HW
    t_changed = (HW - n_ti) / HW
    td_b = (changed - t_changed) ** 2
    pres_p = tot[:, S_NP:S_NP + C] > 0.5
    pres_t = tot[:, S_NT:S_NT + C] > 0.5
    missing_b = (pres_t & ~pres_p).sum(axis=1).astype(np.float64)

    # tie-flagged samples: recompute exactly on host
    cnt = tot[:, S_CNTP] + tot[:, S_CNTT] + tot[:, S_CNTI]
    flagged = np.nonzero(cnt > 3 * HW + 0.5)[0]
    if len(flagged):
        pr = inputs["pred"].reshape(B, C, HW)
        tg = inputs["target"].reshape(B, C, HW)
        ig = inputs["input_grid"].reshape(B, C, HW)
        for b in flagged:
            (sum_ce_w[b], exact_b[b], should_b[b], did_b[b],
             td_b[b], missing_b[b]) = _exact_sample(pr[b], tg[b], ig[b])

    ce_loss = sum_ce_w.sum() / (B * HW) + 0.5 * missing_b.sum()
    exact_bonus = -10.0 * exact_b.mean()
    copy_penalty = 5.0 * (should_b * did_b).mean()
    transform_diff = 2.0 * td_b.mean()
    total = ce_loss + exact_bonus + copy_penalty + transform_diff
    return (np.float32(total), np.float32(ce_loss), np.float32(exact_bonus),
            np.float32(copy_penalty), np.float32(transform_diff),
            np.float32(exact_b.sum()))


def kernel(pred, target, input_grid):
    inputs = {"pred": np.ascontiguousarray(np.asarray(pred), dtype=np.float32),
              "target": np.ascontiguousarray(np.asarray(target), dtype=np.float32),
              "input_grid": np.ascontiguousarray(np.asarray(input_grid),
                                                 dtype=np.float32)}
    stats, _ = _run_device(inputs, trace=False)
    return _aggregate(stats, inputs)
